# Initial kernel scaffold
#
"""Multi-head attention (Keras-style, relu-activated dense projections)
for Trainium2, SPMD across 8 NeuronCores.

Problem (full shapes):
    B, S, D, H = 4, 1024, 1024, 16 ; DH = 64
    qp = relu(q @ Wq + bq); kp = relu(k @ Wk + bk); vp = relu(v @ Wv + bv)
    per head h: scores = qh @ kh^T / 8 ; attn = softmax(scores)
    out = relu(concat_h(attn @ vh) @ Wo + bo)

Sharding: core c = (batch b = c//2, head-group g = c%2). Each core computes
the 8 heads of group g for batch b end-to-end and produces the partial
output projection  attn_out_g @ Wo[g*512:(g+1)*512, :]  (no bias / relu).
Host sums the two partials per batch, adds bo, applies relu.

Per-core dataflow (head pair hp = heads 2hp / 2hp+1):
  - host feeds q[b].T etc so projections contract d on the partition dim.
  - Q/K projections transposed: qpT/kpT [128, 4(hp), 1024(s)]; head 2hp at
    partitions 0:64, head 2hp+1 at 64:128 -> the K=64 score matmuls of a
    pair auto-land in different PE row groups and run concurrently.
  - scores pair writes one [128, 1024] 2-bank PSUM tile; one wide exp (ACT)
    emits ex [128, ut, 1024] bf16 (head A cols 0:512, B 512:1024).
  - attn@v: column-paired bf16 matmuls into nt[0:64] / nt[64:128].
  - softmax denominator: DVE tree-sums ex over ut, two K=128 matmuls with a
    ones column reduce partitions -> Z_A (psum row 0) / Z_B (row 32); a
    masked K=33 matmul broadcasts both to [128, 512]; wide DVE reciprocal +
    a single [128, 512] multiply writes attn_out.
  - output projection: full K=128 accumulating matmuls over head pairs.
  - matmuls in float32r (fp22, full PE rate) except the bf16 attention core.
"""

import numpy as np
from contextlib import ExitStack

import concourse.bass as bass
import concourse.mybir as mybir
import concourse.tile as tile
from concourse import bacc

# ---- constants (hardcoded per the contract; kernel.py must be self-contained)
B, S, D, H = 4, 1024, 1024, 16
DG = 512          # feature slice per core (8 heads)
HL = 8            # heads per core
DH = 64
P = 128
NCORES = 8
NJT = DG // P     # 4 feature tiles == head pairs
NST = S // P      # 8 sequence tiles
NDT = D // P      # 8 contraction tiles for projections
NPC = S // 512    # 2 query chunks of 512

F32 = mybir.dt.float32
F32R = mybir.dt.float32r
BF16 = mybir.dt.bfloat16
AF = mybir.ActivationFunctionType


def _d(ap):
    """View a float32 DRAM AP as float32r so DMAs into f32r tiles type-check.
    (walrus requires fp32r matmul operands to be *produced* as fp32r.)"""
    return ap.bitcast(F32R)


def build_bass():
    nc = bacc.Bacc("TRN2", target_bir_lowering=False, debug=False,
                   num_devices=NCORES)

    xqT = nc.dram_tensor("xqT", [D, S], F32, kind="ExternalInput").ap()
    xkT = nc.dram_tensor("xkT", [D, S], F32, kind="ExternalInput").ap()
    xvT = nc.dram_tensor("xvT", [D, S], F32, kind="ExternalInput").ap()
    wq = nc.dram_tensor("wq", [D, DG], F32, kind="ExternalInput").ap()
    wk = nc.dram_tensor("wk", [D, DG], F32, kind="ExternalInput").ap()
    wv = nc.dram_tensor("wv", [D, DG], F32, kind="ExternalInput").ap()
    bq = nc.dram_tensor("bq", [1, DG], F32, kind="ExternalInput").ap()
    bk = nc.dram_tensor("bk", [1, DG], F32, kind="ExternalInput").ap()
    bv = nc.dram_tensor("bv", [1, DG], F32, kind="ExternalInput").ap()
    wo = nc.dram_tensor("wo", [DG, D], F32, kind="ExternalInput").ap()
    ones_in = nc.dram_tensor("ones", [1, 512], F32, kind="ExternalInput").ap()
    bcm_in = nc.dram_tensor("bcmask", [33, P], F32, kind="ExternalInput").ap()
    out = nc.dram_tensor("out", [S, D], F32, kind="ExternalOutput").ap()

    with tile.TileContext(nc) as tc, ExitStack() as ctx, \
            nc.allow_low_precision(reason="fp32r/bf16 compute is intentional"):
        consts = ctx.enter_context(tc.tile_pool(name="consts", bufs=1))
        xpool = ctx.enter_context(tc.tile_pool(name="xpool", bufs=20))
        wpool = ctx.enter_context(tc.tile_pool(name="wpool", bufs=16))
        qkpool = ctx.enter_context(tc.tile_pool(name="qkpool", bufs=1))
        vpool = ctx.enter_context(tc.tile_pool(name="vpool", bufs=1))
        epool = ctx.enter_context(tc.tile_pool(name="epool", bufs=2))
        aopool = ctx.enter_context(tc.tile_pool(name="aopool", bufs=1))
        t1pool = ctx.enter_context(tc.tile_pool(name="t1pool", bufs=1))
        espool = ctx.enter_context(tc.tile_pool(name="espool", bufs=2))
        rpool = ctx.enter_context(tc.tile_pool(name="rpool", bufs=2))
        outpool = ctx.enter_context(tc.tile_pool(name="outpool", bufs=3))

        psA = ctx.enter_context(tc.tile_pool(name="psA", bufs=2, space="PSUM"))
        psB = ctx.enter_context(tc.tile_pool(name="psB", bufs=2, space="PSUM"))
        psZ = ctx.enter_context(tc.tile_pool(name="psZ", bufs=1, space="PSUM"))
        psD = ctx.enter_context(tc.tile_pool(name="psD", bufs=1, space="PSUM"))

        # --- constants
        ones = consts.tile([P, 512], F32R, tag="ones")
        nc.sync.dma_start(out=ones, in_=_d(ones_in.to_broadcast([P, 512])))
        onescol = consts.tile([P, 1], BF16, tag="onescol")
        nc.vector.memset(onescol, 1.0)
        bcmask = consts.tile([33, P], F32R, tag="bcmask")
        nc.sync.dma_start(out=bcmask, in_=_d(bcm_in))
        # zsb: persistent Z staging rows (0 and 32); fill once with finite
        # values so the masked K=33 broadcast matmul never reads NaNs.
        zsb = consts.tile([33, 512], F32R, tag="zsb")
        nc.sync.dma_start(out=zsb, in_=_d(ones_in.to_broadcast([33, 512])))

        bv_sb = consts.tile([1, DG], F32R, tag="bv")
        nc.sync.dma_start(out=bv_sb, in_=_d(bv))

        # --- transposed projections for Q and K
        qpT = qkpool.tile([P, NJT, S], F32R, tag="qpT")
        kpT = qkpool.tile([P, NJT, S], F32R, tag="kpT")

        # per-partition bias for the transposed projections (ACT bias input)
        bqT = consts.tile([P, NJT], F32, tag="bqT")
        nc.sync.dma_start(out=bqT, in_=bq[0, :].rearrange("(jt p) -> p jt", p=P))
        bkT = consts.tile([P, NJT], F32, tag="bkT")
        nc.sync.dma_start(out=bkT, in_=bk[0, :].rearrange("(jt p) -> p jt", p=P))

        def load_halves(xT, w):
            xmap = {}
            for pc in range(NPC):
                for dt_ in range(NDT):
                    xt = xpool.tile([P, 512], F32R, tag="xT")
                    nc.sync.dma_start(
                        out=xt,
                        in_=_d(xT[dt_ * P:(dt_ + 1) * P,
                                  pc * 512:(pc + 1) * 512]))
                    xmap[(dt_, pc)] = xt
            wts = []
            for dt_ in range(NDT):
                wt = wpool.tile([P, DG], F32R, tag="w")
                nc.sync.dma_start(out=wt, in_=_d(w[dt_ * P:(dt_ + 1) * P, :]))
                wts.append(wt)
            return xmap, wts

        for name, xT, w, bT, dst in (("q", xqT, wq, bqT, qpT),
                                     ("k", xkT, wk, bkT, kpT)):
            xmap, wts = load_halves(xT, w)
            for pc in range(NPC):
                for jt in range(NJT):
                    ps = psA.tile([P, 1024], F32, tag="ps")
                    half = ps[:, 0:512]
                    for dt_ in range(NDT):
                        nc.tensor.matmul(
                            half,
                            lhsT=wts[dt_][:, jt * P:(jt + 1) * P],
                            rhs=xmap[(dt_, pc)],
                            start=(dt_ == 0), stop=(dt_ == NDT - 1))
                    nc.scalar.activation(
                        dst[:, jt, pc * 512:(pc + 1) * 512], half, AF.Relu,
                        bias=bT[:, jt:jt + 1])

        # --- V projection, natural layout -> vpa [128, st, 512] bf16
        vpa = vpool.tile([P, NST, DG], BF16, tag="vpa")
        xmap, wts = load_halves(xvT, wv)
        for st in range(NST):
            ps = psA.tile([P, 1024], F32, tag="ps")
            half = ps[:, 0:512]
            for dt_ in range(NDT):
                nc.tensor.matmul(
                    half,
                    lhsT=xmap[(dt_, st // 4)][:, (st % 4) * P:(st % 4 + 1) * P],
                    rhs=wts[dt_],
                    start=(dt_ == 0), stop=False)
            nc.tensor.matmul(
                half, lhsT=ones[0:1, 0:P], rhs=bv_sb,
                start=False, stop=True)
            nc.scalar.activation(vpa[:, st, :], half, AF.Relu)

        # --- attention, one head pair x one 512-query chunk at a time.
        # pc outer: all head pairs of a query chunk finish together, so the
        # matching half of the output projection can start while the second
        # chunk's attention is still running.
        aoT3 = aopool.tile([P, NJT, S], F32R, tag="aoT3")

        # Wo by head pair (emitted here so its DMA runs during attention)
        wo3 = consts.tile([P, NJT, D], F32R, tag="wo3")
        for hp in range(NJT):
            nc.sync.dma_start(out=wo3[:, hp, :],
                              in_=_d(wo[hp * P:(hp + 1) * P, :]))

        for pc in range(NPC):
            pslice = slice(pc * 512, (pc + 1) * 512)
            for hp in range(NJT):
                hA, hB = 2 * hp, 2 * hp + 1
                ex = epool.tile([P, NST, 1024], BF16, tag="exp")
                for ut in range(NST):
                    uslice = slice(ut * P, (ut + 1) * P)
                    pw = psA.tile([P, 1024], F32, tag="ps")
                    nc.tensor.matmul(
                        pw[:, 0:512],
                        lhsT=kpT[0:DH, hp, uslice],
                        rhs=qpT[0:DH, hp, pslice],
                        start=True, stop=True)
                    nc.tensor.matmul(
                        pw[:, 512:1024],
                        lhsT=kpT[DH:P, hp, uslice],
                        rhs=qpT[DH:P, hp, pslice],
                        start=True, stop=True)
                    nc.scalar.activation(ex[:, ut, :], pw, AF.Exp, scale=0.125)
                # Z tree-sum over ut on DVE (overlaps the attn@v matmuls)
                t1 = t1pool.tile([P, 4, 1024], BF16, tag="t1")
                nc.vector.tensor_add(t1, ex[:, 0:4, :], ex[:, 4:8, :])
                nc.vector.tensor_add(t1[:, 0:2, :], t1[:, 0:2, :],
                                     t1[:, 2:4, :])
                exsum = espool.tile([P, 1024], BF16, tag="exsum")
                nc.vector.tensor_add(exsum, t1[:, 0, :], t1[:, 1, :])
                # Z_A -> psum row 0, Z_B -> psum row 32 (col group 1), then
                # stage into SBUF; emitted before attn@v so the copies are
                # long done when PE reaches the broadcast matmul.
                zps = psZ.tile([P, 512], F32, tag="z")
                nc.tensor.matmul(zps[0:1, :], lhsT=onescol,
                                 rhs=exsum[:, 0:512], start=True, stop=True)
                nc.tensor.matmul(zps[32:33, :], lhsT=onescol,
                                 rhs=exsum[:, 512:1024], start=True, stop=True)
                nc.vector.tensor_copy(zsb[0:1, :], zps[0:1, :])
                nc.vector.tensor_copy(zsb[32:33, :], zps[32:33, :])
                # attn @ v: column-paired accumulation over key tiles
                nt = psB.tile([P, 512], F32, tag="nt")
                for ut in range(NST):
                    nc.tensor.matmul(
                        nt[0:DH, :],
                        lhsT=vpa[:, ut, hA * DH:(hA + 1) * DH],
                        rhs=ex[:, ut, 0:512],
                        start=(ut == 0), stop=(ut == NST - 1),
                        skip_group_check=True)
                    nc.tensor.matmul(
                        nt[DH:P, :],
                        lhsT=vpa[:, ut, hB * DH:(hB + 1) * DH],
                        rhs=ex[:, ut, 512:1024],
                        start=(ut == 0), stop=(ut == NST - 1),
                        skip_group_check=True)
                # broadcast: rows 0:64 <- Z_A, rows 64:128 <- Z_B
                zbc = psZ.tile([P, 512], F32, tag="z")
                nc.tensor.matmul(zbc, lhsT=bcmask, rhs=zsb,
                                 start=True, stop=True)
                rcp = rpool.tile([P, 512], F32, tag="rcp")
                nc.vector.reciprocal_approx_fast(rcp, zbc)
                nc.vector.tensor_mul(aoT3[:, hp, pslice], nt, rcp)

            # output projection for this query chunk (pt = pc*4 .. pc*4+3)
            for pt in range(pc * 4, pc * 4 + 4):
                for jj in range(2):
                    po_ = psD.tile([P, 512], F32, tag="po")
                    for hp in range(NJT):
                        nc.tensor.matmul(
                            po_,
                            lhsT=aoT3[:, hp, pt * P:(pt + 1) * P],
                            rhs=wo3[:, hp, jj * 512:(jj + 1) * 512],
                            start=(hp == 0), stop=(hp == NJT - 1))
                    os_ = outpool.tile([P, 512], F32, tag="os")
                    nc.vector.tensor_copy(os_, po_)
                    nc.sync.dma_start(
                        out=out[pt * P:(pt + 1) * P, jj * 512:(jj + 1) * 512],
                        in_=os_)

    nc.compile()
    return nc


_CACHE = {}


def get_nc():
    if "nc" not in _CACHE:
        _CACHE["nc"] = build_bass()
    return _CACHE["nc"]


def make_bcmask():
    m = np.zeros((33, P), np.float32)
    m[0, 0:DH] = 1.0
    m[32, DH:P] = 1.0
    return m


def make_in_maps(q, k, v, Wq, bq, Wk, bk, Wv, bv, Wo, bo):
    q = np.asarray(q, np.float32)
    k = np.asarray(k, np.float32)
    v = np.asarray(v, np.float32)
    Wq = np.asarray(Wq, np.float32)
    Wk = np.asarray(Wk, np.float32)
    Wv = np.asarray(Wv, np.float32)
    Wo = np.asarray(Wo, np.float32)
    bq = np.asarray(bq, np.float32)
    bk = np.asarray(bk, np.float32)
    bv = np.asarray(bv, np.float32)

    qT = [np.ascontiguousarray(q[b].T) for b in range(B)]
    kT = [np.ascontiguousarray(k[b].T) for b in range(B)]
    vT = [np.ascontiguousarray(v[b].T) for b in range(B)]
    bcm = make_bcmask()

    in_maps = []
    for c in range(NCORES):
        b, g = divmod(c, 2)
        sl = slice(g * DG, (g + 1) * DG)
        in_maps.append({
            "xqT": qT[b],
            "xkT": kT[b],
            "xvT": vT[b],
            "wq": np.ascontiguousarray(Wq[:, sl]),
            "wk": np.ascontiguousarray(Wk[:, sl]),
            "wv": np.ascontiguousarray(Wv[:, sl]),
            "bq": np.ascontiguousarray(bq[sl]).reshape(1, DG),
            "bk": np.ascontiguousarray(bk[sl]).reshape(1, DG),
            "bv": np.ascontiguousarray(bv[sl]).reshape(1, DG),
            "wo": np.ascontiguousarray(Wo[sl, :]),
            "ones": np.ones((1, 512), np.float32),
            "bcmask": bcm,
        })
    return in_maps


def combine_outputs(parts, bo):
    bo = np.asarray(bo, np.float32)
    out = np.empty((B, S, D), np.float32)
    for b in range(B):
        out[b] = np.maximum(parts[2 * b] + parts[2 * b + 1] + bo[None, :], 0.0)
    return out


def run(in_maps, trace=False, **kwargs):
    from concourse.bass_utils import run_bass_kernel_spmd
    nc = get_nc()
    return run_bass_kernel_spmd(nc, in_maps, list(range(NCORES)),
                                trace=trace, **kwargs)


def kernel(q, k, v, Wq, bq, Wk, bk, Wv, bv, Wo, bo):
    in_maps = make_in_maps(q, k, v, Wq, bq, Wk, bk, Wv, bv, Wo, bo)
    res = run(in_maps)
    parts = [res.results[c]["out"] for c in range(NCORES)]
    return combine_outputs(parts, bo)



# revision 1
# speedup vs baseline: 1.0219x; 1.0219x over previous
"""Multi-head attention (Keras-style, relu-activated dense projections)
for Trainium2, SPMD across 8 NeuronCores.

Problem (full shapes):
    B, S, D, H = 4, 1024, 1024, 16 ; DH = 64
    qp = relu(q @ Wq + bq); kp = relu(k @ Wk + bk); vp = relu(v @ Wv + bv)
    per head h: scores = qh @ kh^T / 8 ; attn = softmax(scores)
    out = relu(concat_h(attn @ vh) @ Wo + bo)

Sharding: core c = (batch b = c//2, head-group g = c%2). Each core computes
the 8 heads of group g for batch b end-to-end and produces the partial
output projection  attn_out_g @ Wo[g*512:(g+1)*512, :]  (no bias / relu).
Host sums the two partials per batch, adds bo, applies relu.

Per-core dataflow (head pair hp = heads 2hp / 2hp+1):
  - host feeds q[b].T etc so projections contract d on the partition dim.
  - Q/K projections transposed: qpT/kpT [128, 4(hp), 1024(s)]; head 2hp at
    partitions 0:64, head 2hp+1 at 64:128 -> the K=64 score matmuls of a
    pair auto-land in different PE row groups and run concurrently.
  - scores pair writes one [128, 1024] 2-bank PSUM tile; one wide exp (ACT)
    emits ex [128, ut, 1024] bf16 (head A cols 0:512, B 512:1024).
  - attn@v: column-paired bf16 matmuls into nt[0:64] / nt[64:128].
  - softmax denominator: DVE tree-sums ex over ut, two K=128 matmuls with a
    ones column reduce partitions -> Z_A (psum row 0) / Z_B (row 32); a
    masked K=33 matmul broadcasts both to [128, 512]; wide DVE reciprocal +
    a single [128, 512] multiply writes attn_out.
  - output projection: full K=128 accumulating matmuls over head pairs.
  - matmuls in float32r (fp22, full PE rate) except the bf16 attention core.
"""

import numpy as np
from contextlib import ExitStack

import concourse.bass as bass
import concourse.mybir as mybir
import concourse.tile as tile
from concourse import bacc

# ---- constants (hardcoded per the contract; kernel.py must be self-contained)
B, S, D, H = 4, 1024, 1024, 16
DG = 512          # feature slice per core (8 heads)
HL = 8            # heads per core
DH = 64
P = 128
NCORES = 8
NJT = DG // P     # 4 feature tiles == head pairs
NST = S // P      # 8 sequence tiles
NDT = D // P      # 8 contraction tiles for projections
NPC = S // 512    # 2 query chunks of 512

F32 = mybir.dt.float32
F32R = mybir.dt.float32r
BF16 = mybir.dt.bfloat16
AF = mybir.ActivationFunctionType


def _d(ap):
    """View a float32 DRAM AP as float32r so DMAs into f32r tiles type-check.
    (walrus requires fp32r matmul operands to be *produced* as fp32r.)"""
    return ap.bitcast(F32R)


def build_bass():
    nc = bacc.Bacc("TRN2", target_bir_lowering=False, debug=False,
                   num_devices=NCORES)

    xqT = nc.dram_tensor("xqT", [D, S], F32, kind="ExternalInput").ap()
    xkT = nc.dram_tensor("xkT", [D, S], F32, kind="ExternalInput").ap()
    xvT = nc.dram_tensor("xvT", [D, S], F32, kind="ExternalInput").ap()
    wq = nc.dram_tensor("wq", [D, DG], F32, kind="ExternalInput").ap()
    wk = nc.dram_tensor("wk", [D, DG], F32, kind="ExternalInput").ap()
    wv = nc.dram_tensor("wv", [D, DG], F32, kind="ExternalInput").ap()
    bq = nc.dram_tensor("bq", [1, DG], F32, kind="ExternalInput").ap()
    bk = nc.dram_tensor("bk", [1, DG], F32, kind="ExternalInput").ap()
    bv = nc.dram_tensor("bv", [1, DG], F32, kind="ExternalInput").ap()
    wo = nc.dram_tensor("wo", [DG, D], F32, kind="ExternalInput").ap()
    ones_in = nc.dram_tensor("ones", [1, 512], F32, kind="ExternalInput").ap()
    bcm_in = nc.dram_tensor("bcmask", [33, P], F32, kind="ExternalInput").ap()
    out = nc.dram_tensor("out", [S, D], F32, kind="ExternalOutput").ap()

    with tile.TileContext(nc) as tc, ExitStack() as ctx, \
            nc.allow_low_precision(reason="fp32r/bf16 compute is intentional"):
        consts = ctx.enter_context(tc.tile_pool(name="consts", bufs=1))
        xpool = ctx.enter_context(tc.tile_pool(name="xpool", bufs=20))
        wpool = ctx.enter_context(tc.tile_pool(name="wpool", bufs=16))
        qkpool = ctx.enter_context(tc.tile_pool(name="qkpool", bufs=1))
        vpool = ctx.enter_context(tc.tile_pool(name="vpool", bufs=1))
        epool = ctx.enter_context(tc.tile_pool(name="epool", bufs=2))
        aopool = ctx.enter_context(tc.tile_pool(name="aopool", bufs=1))
        t1pool = ctx.enter_context(tc.tile_pool(name="t1pool", bufs=1))
        espool = ctx.enter_context(tc.tile_pool(name="espool", bufs=2))
        rpool = ctx.enter_context(tc.tile_pool(name="rpool", bufs=2))
        outpool = ctx.enter_context(tc.tile_pool(name="outpool", bufs=3))

        psA = ctx.enter_context(tc.tile_pool(name="psA", bufs=2, space="PSUM"))
        psB = ctx.enter_context(tc.tile_pool(name="psB", bufs=2, space="PSUM"))
        psZ = ctx.enter_context(tc.tile_pool(name="psZ", bufs=1, space="PSUM"))
        psD = ctx.enter_context(tc.tile_pool(name="psD", bufs=1, space="PSUM"))

        # --- constants
        ones = consts.tile([P, 512], F32R, tag="ones")
        nc.sync.dma_start(out=ones, in_=_d(ones_in.to_broadcast([P, 512])))
        onescol = consts.tile([P, 1], BF16, tag="onescol")
        nc.vector.memset(onescol, 1.0)
        bcmask = consts.tile([33, P], F32R, tag="bcmask")
        nc.sync.dma_start(out=bcmask, in_=_d(bcm_in))
        # zsb: persistent Z staging rows (0 and 32); fill once with finite
        # values so the masked K=33 broadcast matmul never reads NaNs.
        zsb = consts.tile([33, 512], F32R, tag="zsb")
        nc.sync.dma_start(out=zsb, in_=_d(ones_in.to_broadcast([33, 512])))

        bv_sb = consts.tile([1, DG], F32R, tag="bv")
        nc.sync.dma_start(out=bv_sb, in_=_d(bv))

        # --- transposed projections for Q and K
        qpT = qkpool.tile([P, NJT, S], F32R, tag="qpT")
        kpT = qkpool.tile([P, NJT, S], F32R, tag="kpT")

        # per-partition bias for the transposed projections (ACT bias input)
        bqT = consts.tile([P, NJT], F32, tag="bqT")
        nc.sync.dma_start(out=bqT, in_=bq[0, :].rearrange("(jt p) -> p jt", p=P))
        bkT = consts.tile([P, NJT], F32, tag="bkT")
        nc.sync.dma_start(out=bkT, in_=bk[0, :].rearrange("(jt p) -> p jt", p=P))

        def load_halves(xT, w):
            xmap = {}
            for pc in range(NPC):
                for dt_ in range(NDT):
                    xt = xpool.tile([P, 512], F32R, tag="xT")
                    nc.sync.dma_start(
                        out=xt,
                        in_=_d(xT[dt_ * P:(dt_ + 1) * P,
                                  pc * 512:(pc + 1) * 512]))
                    xmap[(dt_, pc)] = xt
            wts = []
            for dt_ in range(NDT):
                wt = wpool.tile([P, DG], F32R, tag="w")
                nc.sync.dma_start(out=wt, in_=_d(w[dt_ * P:(dt_ + 1) * P, :]))
                wts.append(wt)
            return xmap, wts

        for name, xT, w, bT, dst in (("q", xqT, wq, bqT, qpT),
                                     ("k", xkT, wk, bkT, kpT)):
            xmap, wts = load_halves(xT, w)
            for pc in range(NPC):
                for jt in range(NJT):
                    ps = psA.tile([P, 1024], F32, tag="ps")
                    half = ps[:, 0:512]
                    for dt_ in range(NDT):
                        nc.tensor.matmul(
                            half,
                            lhsT=wts[dt_][:, jt * P:(jt + 1) * P],
                            rhs=xmap[(dt_, pc)],
                            start=(dt_ == 0), stop=(dt_ == NDT - 1))
                    nc.scalar.activation(
                        dst[:, jt, pc * 512:(pc + 1) * 512], half, AF.Relu,
                        bias=bT[:, jt:jt + 1])

        # --- V projection, natural layout -> vpa [128, st, 512] bf16
        vpa = vpool.tile([P, NST, DG], BF16, tag="vpa")
        xmap, wts = load_halves(xvT, wv)
        for st in range(NST):
            ps = psA.tile([P, 1024], F32, tag="ps")
            half = ps[:, 0:512]
            for dt_ in range(NDT):
                nc.tensor.matmul(
                    half,
                    lhsT=xmap[(dt_, st // 4)][:, (st % 4) * P:(st % 4 + 1) * P],
                    rhs=wts[dt_],
                    start=(dt_ == 0), stop=False)
            nc.tensor.matmul(
                half, lhsT=ones[0:1, 0:P], rhs=bv_sb,
                start=False, stop=True)
            nc.scalar.activation(vpa[:, st, :], half, AF.Relu)

        # --- attention, one head pair x one 512-query chunk at a time.
        # pc outer: all head pairs of a query chunk finish together, so the
        # matching half of the output projection can start while the second
        # chunk's attention is still running.
        aoT3 = aopool.tile([P, NJT, S], F32R, tag="aoT3")

        # Wo by head pair (emitted here so its DMA runs during attention)
        wo3 = consts.tile([P, NJT, D], F32R, tag="wo3")
        for hp in range(NJT):
            nc.sync.dma_start(out=wo3[:, hp, :],
                              in_=_d(wo[hp * P:(hp + 1) * P, :]))

        for pc in range(NPC):
            pslice = slice(pc * 512, (pc + 1) * 512)
            for hp in range(NJT):
                hA, hB = 2 * hp, 2 * hp + 1
                ex = epool.tile([P, NST, 1024], BF16, tag="exp")
                for ut in range(NST):
                    uslice = slice(ut * P, (ut + 1) * P)
                    pw = psA.tile([P, 1024], F32, tag="ps")
                    nc.tensor.matmul(
                        pw[:, 0:512],
                        lhsT=kpT[0:DH, hp, uslice],
                        rhs=qpT[0:DH, hp, pslice],
                        start=True, stop=True)
                    nc.tensor.matmul(
                        pw[:, 512:1024],
                        lhsT=kpT[DH:P, hp, uslice],
                        rhs=qpT[DH:P, hp, pslice],
                        start=True, stop=True)
                    nc.scalar.activation(ex[:, ut, :], pw, AF.Exp, scale=0.125)
                # Z tree-sum over ut on DVE (overlaps the attn@v matmuls)
                t1 = t1pool.tile([P, 4, 1024], BF16, tag="t1")
                nc.vector.tensor_add(t1, ex[:, 0:4, :], ex[:, 4:8, :])
                nc.vector.tensor_add(t1[:, 0:2, :], t1[:, 0:2, :],
                                     t1[:, 2:4, :])
                exsum = espool.tile([P, 1024], BF16, tag="exsum")
                nc.vector.tensor_add(exsum, t1[:, 0, :], t1[:, 1, :])
                # Z_A -> psum row 0, Z_B -> psum row 32 (col group 1), then
                # stage into SBUF; emitted before attn@v so the copies are
                # long done when PE reaches the broadcast matmul.
                zps = psZ.tile([P, 512], F32, tag="z")
                nc.tensor.matmul(zps[0:1, :], lhsT=onescol,
                                 rhs=exsum[:, 0:512], start=True, stop=True)
                nc.tensor.matmul(zps[32:33, :], lhsT=onescol,
                                 rhs=exsum[:, 512:1024], start=True, stop=True)
                nc.vector.tensor_copy(zsb[0:1, :], zps[0:1, :])
                nc.vector.tensor_copy(zsb[32:33, :], zps[32:33, :])
                # attn @ v: column-paired accumulation over key tiles
                nt = psB.tile([P, 512], F32, tag="nt")
                for ut in range(NST):
                    nc.tensor.matmul(
                        nt[0:DH, :],
                        lhsT=vpa[:, ut, hA * DH:(hA + 1) * DH],
                        rhs=ex[:, ut, 0:512],
                        start=(ut == 0), stop=(ut == NST - 1),
                        skip_group_check=True)
                    nc.tensor.matmul(
                        nt[DH:P, :],
                        lhsT=vpa[:, ut, hB * DH:(hB + 1) * DH],
                        rhs=ex[:, ut, 512:1024],
                        start=(ut == 0), stop=(ut == NST - 1),
                        skip_group_check=True)
                # broadcast: rows 0:64 <- Z_A, rows 64:128 <- Z_B
                zbc = psZ.tile([P, 512], F32, tag="z")
                nc.tensor.matmul(zbc, lhsT=bcmask, rhs=zsb,
                                 start=True, stop=True)
                rcp = rpool.tile([P, 512], F32, tag="rcp")
                nc.vector.reciprocal_approx_fast(rcp, zbc)
                nc.vector.tensor_mul(aoT3[:, hp, pslice], nt, rcp)

            # output projection for this query chunk (pt = pc*4 .. pc*4+3)
            for pt in range(pc * 4, pc * 4 + 4):
                for jj in range(2):
                    po_ = psD.tile([P, 512], F32, tag="po")
                    for hp in range(NJT):
                        nc.tensor.matmul(
                            po_,
                            lhsT=aoT3[:, hp, pt * P:(pt + 1) * P],
                            rhs=wo3[:, hp, jj * 512:(jj + 1) * 512],
                            start=(hp == 0), stop=(hp == NJT - 1))
                    os_ = outpool.tile([P, 512], F32, tag="os")
                    nc.vector.tensor_copy(os_, po_)
                    nc.sync.dma_start(
                        out=out[pt * P:(pt + 1) * P, jj * 512:(jj + 1) * 512],
                        in_=os_)

    nc.compile()
    return nc


_CACHE = {}


def get_nc():
    if "nc" not in _CACHE:
        _CACHE["nc"] = build_bass()
    return _CACHE["nc"]


def make_bcmask():
    m = np.zeros((33, P), np.float32)
    m[0, 0:DH] = 1.0
    m[32, DH:P] = 1.0
    return m


def make_in_maps(q, k, v, Wq, bq, Wk, bk, Wv, bv, Wo, bo):
    q = np.asarray(q, np.float32)
    k = np.asarray(k, np.float32)
    v = np.asarray(v, np.float32)
    Wq = np.asarray(Wq, np.float32)
    Wk = np.asarray(Wk, np.float32)
    Wv = np.asarray(Wv, np.float32)
    Wo = np.asarray(Wo, np.float32)
    bq = np.asarray(bq, np.float32)
    bk = np.asarray(bk, np.float32)
    bv = np.asarray(bv, np.float32)

    qT = [np.ascontiguousarray(q[b].T) for b in range(B)]
    kT = [np.ascontiguousarray(k[b].T) for b in range(B)]
    vT = [np.ascontiguousarray(v[b].T) for b in range(B)]
    bcm = make_bcmask()

    in_maps = []
    for c in range(NCORES):
        b, g = divmod(c, 2)
        sl = slice(g * DG, (g + 1) * DG)
        in_maps.append({
            "xqT": qT[b],
            "xkT": kT[b],
            "xvT": vT[b],
            "wq": np.ascontiguousarray(Wq[:, sl]),
            "wk": np.ascontiguousarray(Wk[:, sl]),
            "wv": np.ascontiguousarray(Wv[:, sl]),
            "bq": np.ascontiguousarray(bq[sl]).reshape(1, DG),
            "bk": np.ascontiguousarray(bk[sl]).reshape(1, DG),
            "bv": np.ascontiguousarray(bv[sl]).reshape(1, DG),
            "wo": np.ascontiguousarray(Wo[sl, :]),
            "ones": np.ones((1, 512), np.float32),
            "bcmask": bcm,
        })
    return in_maps


def combine_outputs(parts, bo):
    bo = np.asarray(bo, np.float32)
    out = np.empty((B, S, D), np.float32)
    for b in range(B):
        out[b] = np.maximum(parts[2 * b] + parts[2 * b + 1] + bo[None, :], 0.0)
    return out


def run(in_maps, trace=False, **kwargs):
    from concourse.bass_utils import run_bass_kernel_spmd
    nc = get_nc()
    return run_bass_kernel_spmd(nc, in_maps, list(range(NCORES)),
                                trace=trace, **kwargs)


def kernel(q, k, v, Wq, bq, Wk, bk, Wv, bv, Wo, bo):
    in_maps = make_in_maps(q, k, v, Wq, bq, Wk, bk, Wv, bv, Wo, bo)
    res = run(in_maps)
    parts = [res.results[c]["out"] for c in range(NCORES)]
    return combine_outputs(parts, bo)

